# revision 1
# baseline (speedup 1.0000x reference)
import os

os.environ.setdefault("NEURON_CC_FLAGS", "--auto-cast=none")

import numpy as np
import jax
import jax.numpy as jnp

try:
    jax.config.update("jax_compilation_cache_dir", "/tmp/jax_comp_cache")
    jax.config.update("jax_persistent_cache_min_entry_size_bytes", -1)
    jax.config.update("jax_persistent_cache_min_compile_time_secs", 0.0)
except Exception:
    pass

# Problem: nn_Model_23622320128521 (moe_routing)
# Shapes (hardcoded): Ps=6, B=16, C=8, L=64, D=512, DF=2048, PRED=96, H=8
# Sharding: data-parallel over batch B across 8 cores (2 batches/core).
# Each core runs all 6 experts for its batch slice, then does the
# gate-weighted combine + prediction head locally -> no collectives.

H = 8
EPS = 1e-5
N_CORES = 8

_PARAM_NAMES = [
    "cWq", "cbq", "cWk", "cbk", "cWv", "cbv", "cWo", "cbo",
    "iWq", "ibq", "iWk", "ibk", "iWv", "ibv", "iWo", "ibo",
    "mW1", "mb1", "mW2", "mb2",
    "g1", "b1", "g3", "b3", "g4", "b4",
    "hW", "hb",
]


def _ln(x, g, b):
    m = x.mean(-1, keepdims=True)
    v = ((x - m) ** 2).mean(-1, keepdims=True)
    return (x - m) / jnp.sqrt(v + EPS) * g + b


def _mha(q, k, v):
    Bq, A, S, Dm = q.shape
    dh = Dm // H
    q = q.reshape(Bq, A, S, H, dh)
    k = k.reshape(Bq, A, S, H, dh)
    v = v.reshape(Bq, A, S, H, dh)
    sc = jnp.einsum("bashe,bathe->bahst", q, k) / jnp.sqrt(jnp.asarray(dh, q.dtype))
    a = jax.nn.softmax(sc, axis=-1)
    o = jnp.einsum("bahst,bathe->bashe", a, v)
    return o.reshape(Bq, A, S, Dm)


def _forward(expert_x, gates, p):
    # expert_x: [Ps, b, C, L, D] (local batch slice), gates: [b, Ps]
    def layer(x):
        q = x @ p["cWq"] + p["cbq"]
        k = x @ p["cWk"] + p["cbk"]
        v = x @ p["cWv"] + p["cbv"]
        o = _mha(q, k, v) @ p["cWo"] + p["cbo"]
        x = _ln(x + o, p["g1"], p["b1"])
        q = (x @ p["iWq"] + p["ibq"]).swapaxes(1, 2)
        k = (x @ p["iWk"] + p["ibk"]).swapaxes(1, 2)
        v = (x @ p["iWv"] + p["ibv"]).swapaxes(1, 2)
        o = _mha(q, k, v).swapaxes(1, 2) @ p["iWo"] + p["ibo"]
        x = _ln(x + o, p["g3"], p["b3"])
        h = jnp.maximum(x @ p["mW1"] + p["mb1"], 0.0) @ p["mW2"] + p["mb2"]
        return _ln(x + h, p["g4"], p["b4"])

    enc = jax.vmap(layer)(expert_x)                    # [Ps, b, C, L, D]
    last = enc[:, :, :, -1, :]                         # [Ps, b, C, D]
    combined = jnp.einsum("pbcd,bp->bcd", last, gates)
    out = combined @ p["hW"] + p["hb"]                 # [b, C, PRED]
    return out.transpose(0, 2, 1)                      # [b, PRED, C]


_CACHE = {}


def _get_pmapped():
    if "fn" not in _CACHE:
        devs = [d for d in jax.devices() if d.platform != "cpu"][:N_CORES]
        if len(devs) < N_CORES:
            devs = jax.devices()[:N_CORES]
        _CACHE["fn"] = jax.pmap(_forward, in_axes=(0, 0, None), devices=devs)
    return _CACHE["fn"]


def kernel(**inputs):
    ex = np.asarray(inputs["expert_x"], dtype=np.float32)   # [6,16,8,64,512]
    gates = np.asarray(inputs["gates"], dtype=np.float32)   # [16,6]
    p = {k: jnp.asarray(inputs[k], dtype=np.float32) for k in _PARAM_NAMES}

    B = ex.shape[1]
    per = B // N_CORES
    # [Ps,B,...] -> [N_CORES, Ps, per, ...]
    exs = np.stack(np.split(ex, N_CORES, axis=1), axis=0)
    gs = np.stack(np.split(gates, N_CORES, axis=0), axis=0)  # [8, per, 6]

    fn = _get_pmapped()
    out = fn(exs, gs, p)                     # [8, per, PRED, C]
    out = np.asarray(jax.device_get(out))
    return out.reshape(B, out.shape[2], out.shape[3]).astype(np.float32)

